# revision 11
# baseline (speedup 1.0000x reference)
"""Trainium2 Bass kernel: AdaptiveTokenDictionaryAttention.

Reference computation (per batch b):
    q = x @ Wq + bq                    (n, rd)
    k = td @ Wk + bk                   (m, rd)
    v = td @ Wv + bv                   (m, c)
    qn, kn = l2norm(q), l2norm(k)      (row-wise, eps=1e-12)
    attn = (qn @ kn.T) * (1 + clip(scale,0,1)*log(m))   (n, m)   [output 2]
    x_attn = softmax(attn, -1) @ v     (n, c)                     [output 1]

Strategy: pure data parallel over batch — 16 batches onto 8 NeuronCores
(2 each), weights replicated, no collectives; host splits/concats.

Device-side algebraic refactor (validated ~4.5e-3 rel err vs f32 ref):
    kns  = k * (t[m] / ||k[m]||)         folds temperature into k rows
    W2   = [Wq | Wq @ kns.T]             (c, rd+m), built per batch on PE
    out2 = x @ W2 = [q | raw]            one fused matmul per n-tile
    attn = raw * rsqrt(sum(q^2))         per-partition scalar multiply
    x_attn = (exp(attn)/rowsum) @ v
This keeps every reduction along the free dim, needs no per-n-tile
weight reloads beyond the x tiles themselves, and only one on-chip
transpose per tile (softmax weights for the second matmul).

Host passes x/td pre-transposed ([b, c, n]) and bf16-cast, plus bf16
weight repacks — layout-only transforms so the device never transposes x.

Matmul dtype is bf16 (f32 PSUM accumulation). f32 I/O.
"""

import math
from contextlib import ExitStack
from dataclasses import dataclass

import ml_dtypes
import numpy as np

import concourse.bass as bass
import concourse.tile as tile
from concourse import bacc, mybir
from concourse.bass_utils import run_bass_kernel_spmd
from concourse.hw_specs import get_activation_tables
from concourse.masks import make_identity

FP32 = mybir.dt.float32
BF16 = mybir.dt.bfloat16
AF = mybir.ActivationFunctionType
ALU = mybir.AluOpType
P = 128
N_CORES = 8


@dataclass(frozen=True)
class Cfg:
    b_loc: int  # batches per core
    n: int      # tokens (multiple of 512)
    c: int      # model dim (multiple of 128)
    rd: int     # query/key dim (<=128)
    m: int      # dictionary tokens (<=128)
    use_bias: bool

    @property
    def cc(self):
        return self.c // P


def _covering_act_set(nc) -> int:
    """Index of the ACT LUT set containing every function we use (Square,
    Ln, Exp, Copy, Identity).  Pre-loading it once means the scalar engine
    never reloads tables mid-kernel (~1.3us per reload otherwise)."""
    need = {AF.Exp, AF.Ln, AF.Square, AF.Copy, AF.Identity}
    for i, fset in enumerate(get_activation_tables(nc.m.arch).values()):
        if need <= fset:
            return i
    raise RuntimeError("no covering activation table set")


def build_graph(cfg: Cfg) -> bass.Bass:
    b_loc, n, c, rd, m = cfg.b_loc, cfg.n, cfg.c, cfg.rd, cfg.m
    cc = cfg.cc
    n_slabs = n // 512

    nc = bacc.Bacc("TRN2", target_bir_lowering=False, debug=False,
                   num_devices=N_CORES)

    xT_d = nc.dram_tensor("xT", [b_loc, c, n], BF16, kind="ExternalInput").ap()
    tdT_d = nc.dram_tensor("tdT", [b_loc, c, m], BF16, kind="ExternalInput").ap()
    wqT_d = nc.dram_tensor("WqT", [rd, c], BF16, kind="ExternalInput").ap()
    wqn_d = nc.dram_tensor("Wqn", [P, cc, rd], BF16, kind="ExternalInput").ap()
    wkn_d = nc.dram_tensor("Wkn", [P, cc, rd], BF16, kind="ExternalInput").ap()
    wvn_d = nc.dram_tensor("Wvn", [P, cc, c], BF16, kind="ExternalInput").ap()
    tsc_d = nc.dram_tensor("tsc", [m, 1], FP32, kind="ExternalInput").ap()
    if cfg.use_bias:
        bqr_d = nc.dram_tensor("bqr", [1, rd], FP32, kind="ExternalInput").ap()
        bqc_d = nc.dram_tensor("bqc", [rd, 1], BF16, kind="ExternalInput").ap()
        bkr_d = nc.dram_tensor("bkr", [1, rd], BF16, kind="ExternalInput").ap()
        bvr_d = nc.dram_tensor("bvr", [1, c], BF16, kind="ExternalInput").ap()
    xa_d = nc.dram_tensor("x_attn", [b_loc, n, c], FP32, kind="ExternalOutput").ap()
    at_d = nc.dram_tensor("attn", [b_loc, n, m], FP32, kind="ExternalOutput").ap()

    with tile.TileContext(nc) as tc, ExitStack() as ctx:
        # one covering LUT load up-front; steady state never switches tables
        nc.scalar.add_instruction(mybir.InstLoadActFuncSet(
            name=nc.get_next_instruction_name(), ins=[], outs=[],
            act_func_set_id=_covering_act_set(nc)))
        consts = ctx.enter_context(tc.tile_pool(name="consts", bufs=1))
        p_w2 = ctx.enter_context(tc.tile_pool(name="w2", bufs=2))
        p_v = ctx.enter_context(tc.tile_pool(name="vsb", bufs=2))
        p_x = ctx.enter_context(tc.tile_pool(name="xslab", bufs=3))
        p_small = ctx.enter_context(tc.tile_pool(name="small", bufs=4))
        p_big = ctx.enter_context(tc.tile_pool(name="bigsb", bufs=3))
        ps_main = ctx.enter_context(tc.tile_pool(name="ps_main", bufs=2, space="PSUM"))
        ps_tr = ctx.enter_context(tc.tile_pool(name="ps_tr", bufs=2, space="PSUM"))
        ps_xa = ctx.enter_context(tc.tile_pool(name="ps_xa", bufs=2, space="PSUM"))
        # prep tiles share 2 bank-sized slots (one tag) to stay within the
        # 8-bank PSUM budget alongside the main-loop pools
        ps_prep = ctx.enter_context(tc.tile_pool(name="ps_prep", bufs=2, space="PSUM"))

        # ---- constants / weights (loaded once) ----
        id_bf = consts.tile([P, P], BF16)
        make_identity(nc, id_bf[:])
        wq_sb = consts.tile([P, cc, rd], BF16)
        nc.sync.dma_start(wq_sb[:], wqn_d[:])
        wqT_sb = consts.tile([rd, c], BF16)
        nc.sync.dma_start(wqT_sb[:], wqT_d[:])
        wk_sb = consts.tile([P, cc, rd], BF16)
        nc.sync.dma_start(wk_sb[:], wkn_d[:])
        wv_sb = consts.tile([P, cc, c], BF16)
        nc.sync.dma_start(wv_sb[:], wvn_d[:])
        t_sb = consts.tile([m, 1], FP32)
        nc.sync.dma_start(t_sb[:], tsc_d[:])
        if cfg.use_bias:
            ones_bf = consts.tile([1, P], BF16)
            nc.vector.memset(ones_bf[:], 1.0)
            bqr_sb = consts.tile([1, rd], FP32)
            nc.sync.dma_start(bqr_sb[:], bqr_d[:])
            bqc_sb = consts.tile([rd, 1], BF16)
            nc.sync.dma_start(bqc_sb[:], bqc_d[:])
            bkr_sb = consts.tile([1, rd], BF16)
            nc.sync.dma_start(bkr_sb[:], bkr_d[:])
            bvr_sb = consts.tile([1, c], BF16)
            nc.sync.dma_start(bvr_sb[:], bvr_d[:])

        for b in range(b_loc):
            # ---- per-batch prep: k, v, kns, W2 = [Wq | Wq@kns.T] ----
            tdT_sb = p_small.tile([P, cc, m], BF16, tag="tdT")
            for ci in range(cc):
                nc.sync.dma_start(tdT_sb[:, ci, :], tdT_d[b, ci * P:(ci + 1) * P, :])

            v_ps = ps_prep.tile([P, c], FP32, tag="prep")
            for ci in range(cc):
                last = (ci == cc - 1) and not cfg.use_bias
                nc.tensor.matmul(v_ps[:m], tdT_sb[:, ci, :], wv_sb[:, ci, :],
                                 start=(ci == 0), stop=last)
            if cfg.use_bias:
                nc.tensor.matmul(v_ps[:m], ones_bf[:, :m], bvr_sb[:],
                                 start=False, stop=True)
            v_sb = p_v.tile([m, c], BF16)
            nc.scalar.copy(v_sb[:], v_ps[:m])

            k_ps = ps_prep.tile([P, rd], FP32, tag="prep")
            for ci in range(cc):
                last = (ci == cc - 1) and not cfg.use_bias
                nc.tensor.matmul(k_ps[:m], tdT_sb[:, ci, :], wk_sb[:, ci, :],
                                 start=(ci == 0), stop=last)
            if cfg.use_bias:
                nc.tensor.matmul(k_ps[:m], ones_bf[:, :m], bkr_sb[:],
                                 start=False, stop=True)

            # 1/||k|| = exp(-0.5*ln(sum k^2)) — Ln/Exp/Square share one LUT set
            ksq = p_small.tile([m, rd], BF16, tag="ksq")
            kss = p_small.tile([m, 1], FP32, tag="kss")
            nc.scalar.activation(ksq[:], k_ps[:m], AF.Square, accum_out=kss[:])
            nc.vector.tensor_scalar_max(kss[:], kss[:], 1e-24)
            kln = p_small.tile([m, 1], FP32, tag="kln")
            nc.scalar.activation(kln[:], kss[:], AF.Ln)
            kin = p_small.tile([m, 1], FP32, tag="kin")
            nc.scalar.activation(kin[:], kln[:], AF.Exp, scale=-0.5)
            kfac = p_small.tile([m, 1], FP32, tag="kfac")
            nc.vector.tensor_mul(kfac[:], kin[:], t_sb[:])
            kns = p_small.tile([m, rd], BF16, tag="kns")
            nc.vector.tensor_scalar_mul(kns[:], k_ps[:m], kfac[:])

            knsT_ps = ps_prep.tile([rd, m], BF16, tag="prep")
            nc.tensor.transpose(knsT_ps[:], kns[:], id_bf[:m, :m])
            knsT = p_small.tile([rd, m], BF16, tag="knsT")
            nc.vector.tensor_copy(knsT[:], knsT_ps[:])

            w2 = p_w2.tile([P, cc, rd + m], BF16)
            for ci in range(cc):
                nc.vector.tensor_copy(w2[:, ci, :rd], wq_sb[:, ci, :])
                wqk_ps = ps_prep.tile([P, m], FP32, tag="prep")
                nc.tensor.matmul(wqk_ps[:], wqT_sb[:, ci * P:(ci + 1) * P], knsT[:],
                                 start=True, stop=True)
                nc.vector.tensor_copy(w2[:, ci, rd:], wqk_ps[:])

            if cfg.use_bias:
                bqk_ps = ps_prep.tile([1, m], FP32, tag="prep")
                nc.tensor.matmul(bqk_ps[:], bqc_sb[:], knsT[:], start=True, stop=True)
                b2 = p_small.tile([1, rd + m], BF16, tag="b2")
                nc.vector.tensor_copy(b2[:, :rd], bqr_sb[:])
                nc.vector.tensor_copy(b2[:, rd:], bqk_ps[:])

            # ---- main loop over n ----
            xT_v = xT_d[b].rearrange("(ci p) n -> p ci n", p=P)
            at_v = at_d[b].rearrange("(nt p) m -> p nt m", p=P)
            xa_v = xa_d[b].rearrange("(nt p) c -> p nt c", p=P)
            for j in range(n_slabs):
                xt = p_x.tile([P, cc, 512], BF16)
                nc.sync.dma_start(xt[:], xT_v[:, :, j * 512:(j + 1) * 512])
                ab4 = p_big.tile([P, 4, m], FP32, tag="ab4")
                for s in range(4):
                    out2 = ps_main.tile([P, rd + m], FP32)
                    for ci in range(cc):
                        last = (ci == cc - 1) and not cfg.use_bias
                        nc.tensor.matmul(out2[:], xt[:, ci, s * P:(s + 1) * P],
                                         w2[:, ci, :], start=(ci == 0), stop=last)
                    if cfg.use_bias:
                        nc.tensor.matmul(out2[:], ones_bf[:], b2[:],
                                         start=False, stop=True)

                    # 1/||q|| = exp(-0.5*ln(sum q^2)); single LUT set with Exp
                    qsq = p_small.tile([P, rd], BF16, tag="qsq")
                    qss = p_small.tile([P, 1], FP32, tag="qss")
                    nc.scalar.activation(qsq[:], out2[:, :rd], AF.Square,
                                         accum_out=qss[:])
                    nc.vector.tensor_scalar_max(qss[:], qss[:], 1e-24)
                    qln = p_small.tile([P, 1], FP32, tag="qln")
                    nc.scalar.activation(qln[:], qss[:], AF.Ln)
                    qin = p_small.tile([P, 1], FP32, tag="qin")
                    nc.scalar.activation(qin[:], qln[:], AF.Exp, scale=-0.5)

                    attn_sb = ab4[:, s, :]
                    nc.vector.tensor_scalar_mul(attn_sb, out2[:, rd:], qin[:])

                    ew = p_small.tile([P, m], BF16, tag="ew")
                    rs = p_small.tile([P, 1], FP32, tag="rs")
                    nc.scalar.activation(ew[:], attn_sb, AF.Exp, accum_out=rs[:])
                    rsi = p_small.tile([P, 1], FP32, tag="rsi")
                    nc.vector.reciprocal(rsi[:], rs[:])
                    ews = p_small.tile([P, m], BF16, tag="ews")
                    nc.vector.tensor_scalar_mul(ews[:], ew[:], rsi[:])

                    ewT_ps = ps_tr.tile([m, P], BF16)
                    nc.tensor.transpose(ewT_ps[:], ews[:], id_bf[:])
                    ewT = p_small.tile([m, P], BF16, tag="ewT")
                    nc.vector.tensor_copy(ewT[:], ewT_ps[:])

                    xa_ps = ps_xa.tile([P, c], FP32)
                    nc.tensor.matmul(xa_ps[:], ewT[:], v_sb[:], start=True, stop=True)
                    if s % 2 == 0:
                        xa2 = p_big.tile([P, 2, c], FP32, tag="xa2")
                        nc.scalar.copy(xa2[:, 0, :], xa_ps[:])
                    else:
                        nc.vector.tensor_copy(xa2[:, 1, :], xa_ps[:])
                        nc.gpsimd.dma_start(
                            xa_v[:, j * 4 + s - 1:j * 4 + s + 1, :], xa2[:])
                nc.gpsimd.dma_start(at_v[:, j * 4:(j + 1) * 4, :], ab4[:])

    nc.finalize()
    return nc


_GRAPH_CACHE: dict = {}


def _get_graph(cfg: Cfg) -> bass.Bass:
    if cfg not in _GRAPH_CACHE:
        _GRAPH_CACHE[cfg] = build_graph(cfg)
    return _GRAPH_CACHE[cfg]


def _prep_host_inputs(x, td, Wq, bq, Wk, bk, Wv, bv, scale, use_bias):
    bf = ml_dtypes.bfloat16
    B, N, C = x.shape
    M = td.shape[1]
    RD = Wq.shape[1]
    cc = C // P
    xT = np.ascontiguousarray(x.transpose(0, 2, 1)).astype(bf)
    tdT = np.ascontiguousarray(td.transpose(0, 2, 1)).astype(bf)
    wqT = np.ascontiguousarray(Wq.T).astype(bf)
    wqn = np.ascontiguousarray(Wq.reshape(cc, P, RD).transpose(1, 0, 2)).astype(bf)
    wkn = np.ascontiguousarray(Wk.reshape(cc, P, RD).transpose(1, 0, 2)).astype(bf)
    wvn = np.ascontiguousarray(Wv.reshape(cc, P, C).transpose(1, 0, 2)).astype(bf)
    t = (1.0 + np.clip(scale, 0.0, 1.0) * np.float32(math.log(M)))
    t = np.ascontiguousarray(t.astype(np.float32).reshape(M, 1))
    common = {"WqT": wqT, "Wqn": wqn, "Wkn": wkn, "Wvn": wvn, "tsc": t}
    if use_bias:
        common["bqr"] = np.ascontiguousarray(bq.reshape(1, RD).astype(np.float32))
        common["bqc"] = np.ascontiguousarray(bq.reshape(RD, 1).astype(bf))
        common["bkr"] = np.ascontiguousarray(bk.reshape(1, RD).astype(bf))
        common["bvr"] = np.ascontiguousarray(bv.reshape(1, C).astype(bf))
    return xT, tdT, common


def kernel_ex(x, td, Wq, bq, Wk, bk, Wv, bv, scale, h=None, w=None,
              trace=False, **_unused):
    """Like kernel(), but also returns the BassKernelResults (for tracing)."""
    x = np.asarray(x, dtype=np.float32)
    td = np.asarray(td, dtype=np.float32)
    Wq = np.asarray(Wq, dtype=np.float32)
    bq = np.asarray(bq, dtype=np.float32)
    Wk = np.asarray(Wk, dtype=np.float32)
    bk = np.asarray(bk, dtype=np.float32)
    Wv = np.asarray(Wv, dtype=np.float32)
    bv = np.asarray(bv, dtype=np.float32)
    scale = np.asarray(scale, dtype=np.float32)

    B, N, C = x.shape
    M = td.shape[1]
    RD = Wq.shape[1]
    assert B % N_CORES == 0
    b_loc = B // N_CORES
    use_bias = bool(np.any(bq) or np.any(bk) or np.any(bv))

    cfg = Cfg(b_loc=b_loc, n=N, c=C, rd=RD, m=M, use_bias=use_bias)
    nc = _get_graph(cfg)

    xT, tdT, common = _prep_host_inputs(x, td, Wq, bq, Wk, bk, Wv, bv, scale,
                                        use_bias)
    in_maps = []
    for cid in range(N_CORES):
        sl = slice(cid * b_loc, (cid + 1) * b_loc)
        im = dict(common)
        im["xT"] = xT[sl]
        im["tdT"] = tdT[sl]
        in_maps.append(im)

    res = run_bass_kernel_spmd(nc, in_maps, core_ids=list(range(N_CORES)),
                               trace=trace)
    x_attn = np.concatenate([r["x_attn"] for r in res.results], axis=0)
    attn = np.concatenate([r["attn"] for r in res.results], axis=0)
    return (x_attn, attn), res


def kernel(*args, **kwargs):
    out, _ = kernel_ex(*args, **kwargs)
    return out


# revision 15
# speedup vs baseline: 1.5884x; 1.5884x over previous
"""Trainium2 Bass kernel: AdaptiveTokenDictionaryAttention.

Reference computation (per batch b):
    q = x @ Wq + bq                    (n, rd)
    k = td @ Wk + bk                   (m, rd)
    v = td @ Wv + bv                   (m, c)
    qn, kn = l2norm(q), l2norm(k)      (row-wise, eps=1e-12)
    attn = (qn @ kn.T) * (1 + clip(scale,0,1)*log(m))   (n, m)   [output 2]
    x_attn = softmax(attn, -1) @ v     (n, c)                     [output 1]

Strategy: pure data parallel over batch — 16 batches onto 8 NeuronCores
(2 each), weights replicated, no collectives; host splits/concats.

Device-side algebraic refactor (validated ~4.5e-3 rel err vs f32 ref):
    kns  = k * (t[m] / ||k[m]||)         folds temperature into k rows
    W2   = [Wq | Wq @ kns.T]             (c, rd+m), built per batch on PE
    out2 = x @ W2 = [q | raw]            one fused matmul per n-tile
    attn = raw * rsqrt(sum(q^2))         per-partition scalar multiply
    x_attn = (exp(attn)/rowsum) @ v
This keeps every reduction along the free dim, needs no per-n-tile
weight reloads beyond the x tiles themselves, and only one on-chip
transpose per tile (softmax weights for the second matmul).

Host passes x/td pre-transposed ([b, c, n]) and bf16-cast, plus bf16
weight repacks — layout-only transforms so the device never transposes x.

Matmul dtype is bf16 (f32 PSUM accumulation). f32 I/O.
"""

import math
from contextlib import ExitStack
from dataclasses import dataclass

import ml_dtypes
import numpy as np

import concourse.bass as bass
import concourse.tile as tile
from concourse import bacc, mybir
from concourse.bass_utils import run_bass_kernel_spmd
from concourse.hw_specs import get_activation_tables
from concourse.masks import make_identity

FP32 = mybir.dt.float32
BF16 = mybir.dt.bfloat16
AF = mybir.ActivationFunctionType
ALU = mybir.AluOpType
P = 128
N_CORES = 8


@dataclass(frozen=True)
class Cfg:
    b_loc: int  # batches per core
    n: int      # tokens (multiple of 512)
    c: int      # model dim (multiple of 128)
    rd: int     # query/key dim (<=128)
    m: int      # dictionary tokens (<=128)
    use_bias: bool

    @property
    def cc(self):
        return self.c // P


def _covering_act_set(nc) -> int:
    """Index of the ACT LUT set containing every function we use (Square,
    Ln, Exp, Copy, Identity).  Pre-loading it once means the scalar engine
    never reloads tables mid-kernel (~1.3us per reload otherwise)."""
    need = {AF.Exp, AF.Ln, AF.Square, AF.Copy, AF.Identity}
    for i, fset in enumerate(get_activation_tables(nc.m.arch).values()):
        if need <= fset:
            return i
    raise RuntimeError("no covering activation table set")


def build_graph(cfg: Cfg) -> bass.Bass:
    b_loc, n, c, rd, m = cfg.b_loc, cfg.n, cfg.c, cfg.rd, cfg.m
    cc = cfg.cc
    n_slabs = n // 512

    nc = bacc.Bacc("TRN2", target_bir_lowering=False, debug=False,
                   num_devices=N_CORES)

    xT_d = nc.dram_tensor("xT", [b_loc, c, n], BF16, kind="ExternalInput").ap()
    tdT_d = nc.dram_tensor("tdT", [b_loc, c, m], BF16, kind="ExternalInput").ap()
    wqT_d = nc.dram_tensor("WqT", [rd, c], BF16, kind="ExternalInput").ap()
    wqn_d = nc.dram_tensor("Wqn", [P, cc, rd], BF16, kind="ExternalInput").ap()
    wkn_d = nc.dram_tensor("Wkn", [P, cc, rd], BF16, kind="ExternalInput").ap()
    wvn_d = nc.dram_tensor("Wvn", [P, cc, c], BF16, kind="ExternalInput").ap()
    tsc_d = nc.dram_tensor("tsc", [m, 1], FP32, kind="ExternalInput").ap()
    if cfg.use_bias:
        bqr_d = nc.dram_tensor("bqr", [1, rd], FP32, kind="ExternalInput").ap()
        bqc_d = nc.dram_tensor("bqc", [rd, 1], BF16, kind="ExternalInput").ap()
        bkr_d = nc.dram_tensor("bkr", [1, rd], BF16, kind="ExternalInput").ap()
        bvr_d = nc.dram_tensor("bvr", [1, c], BF16, kind="ExternalInput").ap()
    xa_d = nc.dram_tensor("x_attn", [b_loc, n, c], FP32, kind="ExternalOutput").ap()
    at_d = nc.dram_tensor("attn", [b_loc, n, m], FP32, kind="ExternalOutput").ap()

    with tile.TileContext(nc) as tc, ExitStack() as ctx:
        # one covering LUT load up-front; steady state never switches tables
        nc.scalar.add_instruction(mybir.InstLoadActFuncSet(
            name=nc.get_next_instruction_name(), ins=[], outs=[],
            act_func_set_id=_covering_act_set(nc)))
        consts = ctx.enter_context(tc.tile_pool(name="consts", bufs=1))
        p_w2 = ctx.enter_context(tc.tile_pool(name="w2", bufs=2))
        p_v = ctx.enter_context(tc.tile_pool(name="vsb", bufs=2))
        p_x = ctx.enter_context(tc.tile_pool(name="xslab", bufs=3))
        p_small = ctx.enter_context(tc.tile_pool(name="small", bufs=4))
        p_big = ctx.enter_context(tc.tile_pool(name="bigsb", bufs=3))
        # PSUM budget is 8 banks: out2 x4 (slab-batched norms keep 4 alive),
        # transpose x2, xattn x2; per-batch prep tiles share the xattn slots.
        ps_main = ctx.enter_context(tc.tile_pool(name="ps_main", bufs=4, space="PSUM"))
        ps_tr = ctx.enter_context(tc.tile_pool(name="ps_tr", bufs=2, space="PSUM"))
        ps_xa = ctx.enter_context(tc.tile_pool(name="ps_xa", bufs=2, space="PSUM"))

        # ---- constants / weights (loaded once) ----
        id_bf = consts.tile([P, P], BF16)
        make_identity(nc, id_bf[:])
        wq_sb = consts.tile([P, cc, rd], BF16)
        nc.sync.dma_start(wq_sb[:], wqn_d[:])
        wqT_sb = consts.tile([rd, c], BF16)
        nc.sync.dma_start(wqT_sb[:], wqT_d[:])
        wk_sb = consts.tile([P, cc, rd], BF16)
        nc.sync.dma_start(wk_sb[:], wkn_d[:])
        wv_sb = consts.tile([P, cc, c], BF16)
        nc.sync.dma_start(wv_sb[:], wvn_d[:])
        t_sb = consts.tile([m, 1], FP32)
        nc.sync.dma_start(t_sb[:], tsc_d[:])
        if cfg.use_bias:
            ones_bf = consts.tile([1, P], BF16)
            nc.vector.memset(ones_bf[:], 1.0)
            bqr_sb = consts.tile([1, rd], FP32)
            nc.sync.dma_start(bqr_sb[:], bqr_d[:])
            bqc_sb = consts.tile([rd, 1], BF16)
            nc.sync.dma_start(bqc_sb[:], bqc_d[:])
            bkr_sb = consts.tile([1, rd], BF16)
            nc.sync.dma_start(bkr_sb[:], bkr_d[:])
            bvr_sb = consts.tile([1, c], BF16)
            nc.sync.dma_start(bvr_sb[:], bvr_d[:])

        for b in range(b_loc):
            # ---- per-batch prep: k, v, kns, W2 = [Wq | Wq@kns.T] ----
            tdT_sb = p_small.tile([P, cc, m], BF16, tag="tdT")
            for ci in range(cc):
                nc.sync.dma_start(tdT_sb[:, ci, :], tdT_d[b, ci * P:(ci + 1) * P, :])

            v_ps = ps_xa.tile([P, c], FP32, tag="xa_ps")
            for ci in range(cc):
                last = (ci == cc - 1) and not cfg.use_bias
                nc.tensor.matmul(v_ps[:m], tdT_sb[:, ci, :], wv_sb[:, ci, :],
                                 start=(ci == 0), stop=last)
            if cfg.use_bias:
                nc.tensor.matmul(v_ps[:m], ones_bf[:, :m], bvr_sb[:],
                                 start=False, stop=True)
            v_sb = p_v.tile([m, c], BF16)
            nc.scalar.copy(v_sb[:], v_ps[:m])

            k_ps = ps_xa.tile([P, rd], FP32, tag="xa_ps")
            for ci in range(cc):
                last = (ci == cc - 1) and not cfg.use_bias
                nc.tensor.matmul(k_ps[:m], tdT_sb[:, ci, :], wk_sb[:, ci, :],
                                 start=(ci == 0), stop=last)
            if cfg.use_bias:
                nc.tensor.matmul(k_ps[:m], ones_bf[:, :m], bkr_sb[:],
                                 start=False, stop=True)

            # 1/||k|| = exp(-0.5*ln(sum k^2)) — Ln/Exp/Square share one LUT set
            ksq = p_small.tile([m, rd], BF16, tag="ksq")
            kss = p_small.tile([m, 1], FP32, tag="kss")
            nc.scalar.activation(ksq[:], k_ps[:m], AF.Square, accum_out=kss[:])
            nc.vector.tensor_scalar_max(kss[:], kss[:], 1e-24)
            kln = p_small.tile([m, 1], FP32, tag="kln")
            nc.scalar.activation(kln[:], kss[:], AF.Ln)
            kin = p_small.tile([m, 1], FP32, tag="kin")
            nc.scalar.activation(kin[:], kln[:], AF.Exp, scale=-0.5)
            kfac = p_small.tile([m, 1], FP32, tag="kfac")
            nc.vector.tensor_mul(kfac[:], kin[:], t_sb[:])
            kns = p_small.tile([m, rd], BF16, tag="kns")
            nc.vector.tensor_scalar_mul(kns[:], k_ps[:m], kfac[:])

            knsT_ps = ps_xa.tile([rd, m], BF16, tag="xa_ps")
            nc.tensor.transpose(knsT_ps[:], kns[:], id_bf[:m, :m])
            knsT = p_small.tile([rd, m], BF16, tag="knsT")
            nc.vector.tensor_copy(knsT[:], knsT_ps[:])

            w2 = p_w2.tile([P, cc, rd + m], BF16)
            for ci in range(cc):
                nc.vector.tensor_copy(w2[:, ci, :rd], wq_sb[:, ci, :])
                wqk_ps = ps_xa.tile([P, m], FP32, tag="xa_ps")
                nc.tensor.matmul(wqk_ps[:], wqT_sb[:, ci * P:(ci + 1) * P], knsT[:],
                                 start=True, stop=True)
                nc.vector.tensor_copy(w2[:, ci, rd:], wqk_ps[:])

            if cfg.use_bias:
                bqk_ps = ps_xa.tile([1, m], FP32, tag="xa_ps")
                nc.tensor.matmul(bqk_ps[:], bqc_sb[:], knsT[:], start=True, stop=True)
                b2 = p_small.tile([1, rd + m], BF16, tag="b2")
                nc.vector.tensor_copy(b2[:, :rd], bqr_sb[:])
                nc.vector.tensor_copy(b2[:, rd:], bqk_ps[:])

            # ---- main loop over n ----
            xT_v = xT_d[b].rearrange("(ci p) n -> p ci n", p=P)
            at_v = at_d[b].rearrange("(nt p) m -> p nt m", p=P)
            xa_v = xa_d[b].rearrange("(nt p) c -> p nt c", p=P)
            for j in range(n_slabs):
                xt = p_x.tile([P, cc, 512], BF16)
                nc.sync.dma_start(xt[:], xT_v[:, :, j * 512:(j + 1) * 512])
                ab4 = p_big.tile([P, 4, m], FP32, tag="ab4")
                qs4 = p_small.tile([P, 4], FP32, tag="qs4")
                qi4 = p_small.tile([P, 4], FP32, tag="qi4")
                outs = []
                for s in range(4):
                    out2 = ps_main.tile([P, rd + m], FP32)
                    outs.append(out2)
                    for ci in range(cc):
                        last = (ci == cc - 1) and not cfg.use_bias
                        nc.tensor.matmul(out2[:], xt[:, ci, s * P:(s + 1) * P],
                                         w2[:, ci, :], start=(ci == 0), stop=last)
                    if cfg.use_bias:
                        nc.tensor.matmul(out2[:], ones_bf[:], b2[:],
                                         start=False, stop=True)
                    # sum(q^2) on DVE (PSUM allows only one tensor operand:
                    # stage q in SBUF, then square+accumulate there)
                    qcp = p_small.tile([P, rd], BF16, tag="qcp")
                    nc.vector.tensor_copy(qcp[:], out2[:, :rd])
                    qsq = p_small.tile([P, rd], BF16, tag="qsq")
                    nc.vector.scalar_tensor_tensor(
                        qsq[:], qcp[:], 1.0, qcp[:],
                        op0=ALU.mult, op1=ALU.mult,
                        accum_out=qs4[:, s:s + 1])
                # slab-batched 1/||q|| = exp(-0.5*ln(qs4))
                nc.vector.tensor_scalar_max(qs4[:], qs4[:], 1e-24)
                ql4 = p_small.tile([P, 4], FP32, tag="ql4")
                nc.scalar.activation(ql4[:], qs4[:], AF.Ln)
                nc.scalar.activation(qi4[:], ql4[:], AF.Exp, scale=-0.5)

                for s in range(4):
                    out2 = outs[s]
                    qin = qi4[:, s:s + 1]
                    # attn output tile (raw * 1/||q||)
                    nc.vector.tensor_scalar_mul(ab4[:, s, :], out2[:, rd:], qin)
                    # softmax numerator straight from PSUM: exp(raw * 1/||q||)
                    ew = p_small.tile([P, m], BF16, tag="ew")
                    nc.scalar.activation(ew[:], out2[:, rd:], AF.Exp, scale=qin)
                    rs = p_small.tile([P, 1], FP32, tag="rs")
                    nc.vector.tensor_reduce(rs[:], ew[:], mybir.AxisListType.X,
                                            ALU.add)
                    rsi = p_small.tile([P, 1], FP32, tag="rsi")
                    nc.vector.reciprocal(rsi[:], rs[:])

                    ewT_ps = ps_tr.tile([m, P], BF16)
                    nc.tensor.transpose(ewT_ps[:], ew[:], id_bf[:])
                    ewT = p_small.tile([m, P], BF16, tag="ewT")
                    nc.scalar.copy(ewT[:], ewT_ps[:])

                    xa_ps = ps_xa.tile([P, c], FP32)
                    nc.tensor.matmul(xa_ps[:], ewT[:], v_sb[:], start=True, stop=True)
                    # PSUM->SBUF with the 1/rowsum fold; alternate engines
                    if s % 2 == 0:
                        xa2 = p_big.tile([P, 2, c], FP32, tag="xa2")
                        nc.scalar.activation(xa2[:, 0, :], xa_ps[:], AF.Copy,
                                             scale=rsi[:])
                    else:
                        nc.vector.tensor_scalar_mul(xa2[:, 1, :], xa_ps[:], rsi[:])
                        nc.sync.dma_start(
                            xa_v[:, j * 4 + s - 1:j * 4 + s + 1, :], xa2[:])
                nc.sync.dma_start(at_v[:, j * 4:(j + 1) * 4, :], ab4[:])

    nc.finalize()
    return nc


_GRAPH_CACHE: dict = {}


def _get_graph(cfg: Cfg) -> bass.Bass:
    if cfg not in _GRAPH_CACHE:
        _GRAPH_CACHE[cfg] = build_graph(cfg)
    return _GRAPH_CACHE[cfg]


def _prep_host_inputs(x, td, Wq, bq, Wk, bk, Wv, bv, scale, use_bias):
    bf = ml_dtypes.bfloat16
    B, N, C = x.shape
    M = td.shape[1]
    RD = Wq.shape[1]
    cc = C // P
    xT = np.ascontiguousarray(x.transpose(0, 2, 1)).astype(bf)
    tdT = np.ascontiguousarray(td.transpose(0, 2, 1)).astype(bf)
    wqT = np.ascontiguousarray(Wq.T).astype(bf)
    wqn = np.ascontiguousarray(Wq.reshape(cc, P, RD).transpose(1, 0, 2)).astype(bf)
    wkn = np.ascontiguousarray(Wk.reshape(cc, P, RD).transpose(1, 0, 2)).astype(bf)
    wvn = np.ascontiguousarray(Wv.reshape(cc, P, C).transpose(1, 0, 2)).astype(bf)
    t = (1.0 + np.clip(scale, 0.0, 1.0) * np.float32(math.log(M)))
    t = np.ascontiguousarray(t.astype(np.float32).reshape(M, 1))
    common = {"WqT": wqT, "Wqn": wqn, "Wkn": wkn, "Wvn": wvn, "tsc": t}
    if use_bias:
        common["bqr"] = np.ascontiguousarray(bq.reshape(1, RD).astype(np.float32))
        common["bqc"] = np.ascontiguousarray(bq.reshape(RD, 1).astype(bf))
        common["bkr"] = np.ascontiguousarray(bk.reshape(1, RD).astype(bf))
        common["bvr"] = np.ascontiguousarray(bv.reshape(1, C).astype(bf))
    return xT, tdT, common


def kernel_ex(x, td, Wq, bq, Wk, bk, Wv, bv, scale, h=None, w=None,
              trace=False, **_unused):
    """Like kernel(), but also returns the BassKernelResults (for tracing)."""
    x = np.asarray(x, dtype=np.float32)
    td = np.asarray(td, dtype=np.float32)
    Wq = np.asarray(Wq, dtype=np.float32)
    bq = np.asarray(bq, dtype=np.float32)
    Wk = np.asarray(Wk, dtype=np.float32)
    bk = np.asarray(bk, dtype=np.float32)
    Wv = np.asarray(Wv, dtype=np.float32)
    bv = np.asarray(bv, dtype=np.float32)
    scale = np.asarray(scale, dtype=np.float32)

    B, N, C = x.shape
    M = td.shape[1]
    RD = Wq.shape[1]
    assert B % N_CORES == 0
    b_loc = B // N_CORES
    use_bias = bool(np.any(bq) or np.any(bk) or np.any(bv))

    cfg = Cfg(b_loc=b_loc, n=N, c=C, rd=RD, m=M, use_bias=use_bias)
    nc = _get_graph(cfg)

    xT, tdT, common = _prep_host_inputs(x, td, Wq, bq, Wk, bk, Wv, bv, scale,
                                        use_bias)
    in_maps = []
    for cid in range(N_CORES):
        sl = slice(cid * b_loc, (cid + 1) * b_loc)
        im = dict(common)
        im["xT"] = xT[sl]
        im["tdT"] = tdT[sl]
        in_maps.append(im)

    res = run_bass_kernel_spmd(nc, in_maps, core_ids=list(range(N_CORES)),
                               trace=trace)
    x_attn = np.concatenate([r["x_attn"] for r in res.results], axis=0)
    attn = np.concatenate([r["attn"] for r in res.results], axis=0)
    return (x_attn, attn), res


def kernel(*args, **kwargs):
    out, _ = kernel_ex(*args, **kwargs)
    return out
